# revision 8
# baseline (speedup 1.0000x reference)
"""Trainium2 Bass kernel for nn_DynamicsBase: multi-type one-hot scatter.

Computes out[f, a, 16*t + actions[f, t, a]] = 1.0 over a zero base of shape
[2048, 256, 128] f32. Frames are sharded across 8 NeuronCores (pure data
parallelism, no communication). On each core the one-hot rows are produced
by a DVE tensor_tensor is_equal against an iota constant using broadcast
(step-0) access patterns — one compare per output element, no materialized
broadcast — and streamed to HBM with ~1 MiB DMA stores (the HBM-write
roofline, ~32 MiB/core, is the bottleneck; DVE compute hides under it).

Self-contained: hardcodes shapes; takes full inputs, returns full output.
"""
import numpy as np
from contextlib import ExitStack

import concourse.bass as bass
import concourse.tile as tile
import concourse.mybir as mybir

NUM_FRAMES, NUM_TYPES, NUM_ACTIONS = 2048, 8, 256
J = 16                      # sub-actions per type
TOTAL = NUM_TYPES * J       # 128 one-hot width
N_CORES = 8
F_PER_CORE = NUM_FRAMES // N_CORES  # 256

_CACHE = {}


def _build_nc(FB=16, out_bufs=4):
    nc = bass.Bass()
    act = nc.declare_dram_parameter(
        "actions_t", [2, 128, F_PER_CORE, NUM_TYPES], mybir.dt.uint8,
        isOutput=False)
    out = nc.declare_dram_parameter(
        "out", [F_PER_CORE, NUM_ACTIONS, TOTAL], mybir.dt.float32,
        isOutput=True)

    with ExitStack() as ctx:
        tc = ctx.enter_context(tile.TileContext(nc))
        const_pool = ctx.enter_context(tc.tile_pool(name="const", bufs=1))
        act_pool = ctx.enter_context(tc.tile_pool(name="act", bufs=1))
        out_pool = ctx.enter_context(tc.tile_pool(name="out", bufs=out_bufs))

        # cmod[p, j] = j, built with 16 tiny DVE memsets (no DMA slot used;
        # same-engine program order makes it ready before the first compare)
        cmod_sb = const_pool.tile([128, J], mybir.dt.uint8, name="cmod_sb")
        for j in range(J):
            nc.vector.memset(cmod_sb[:, j:j + 1], j)

        act_sb = [
            act_pool.tile([128, F_PER_CORE * NUM_TYPES], mybir.dt.uint8,
                          name=f"act_sb{h}", tag=f"act{h}")
            for h in range(2)
        ]
        for h in range(2):
            nc.sync.dma_start(act_sb[h][:], act[h].rearrange("a f t -> a (f t)"))

        for h in range(2):
            for fb in range(0, F_PER_CORE, FB):
                o = out_pool.tile([128, FB * TOTAL], mybir.dt.float32,
                                  name=f"o_{h}_{fb}", tag="o")
                in1 = (act_sb[h][:, fb * NUM_TYPES:(fb + FB) * NUM_TYPES]
                       .unsqueeze(2).broadcast_to([128, FB * NUM_TYPES, J]))
                in0 = (cmod_sb[:, :].unsqueeze(1)
                       .broadcast_to([128, FB * NUM_TYPES, J]))
                o_ap = o[:, :].rearrange("p (ft j) -> p ft j", j=J)
                nc.vector.tensor_tensor(o_ap, in0, in1,
                                        op=mybir.AluOpType.is_equal)
                dst = out[fb:fb + FB, h * 128:(h + 1) * 128, :].transpose(
                    [1, 0, 2])
                nc.sync.dma_start(dst, o[:, :].rearrange("p (f c) -> p f c",
                                                         c=TOTAL))
    return nc


def _split_multi_waits(nc):
    """Walrus codegen in this toolchain accepts at most ONE sync-wait per
    instruction ("Too many sync wait commands"). Tile's sem assignment can
    attach 2+. Split the extras onto same-engine NoOps placed just before
    the instruction (program order on the engine preserves semantics)."""
    def fix_block(bb):
        new = []
        for inst in bb.instructions:
            if getattr(inst, "blocks", None):
                for sub in inst.blocks:
                    fix_block(sub)
            si = inst.sync_info
            if si is not None and si.on_wait and len(si.on_wait) > 1:
                waits = list(si.on_wait)
                for k, w in enumerate(waits[:-1]):
                    nop = mybir.InstNoOp(
                        name=f"{inst.name}-waitsplit{k}",
                        engine=inst.engine,
                        ins=[], outs=[],
                        sync_info=mybir.SyncInfo(on_wait=[w], on_update=[]),
                    )
                    nc.register_instruction(nop)
                    new.append(nop)
                si.on_wait = [waits[-1]]
            new.append(inst)
        bb.instructions[:] = new
    for f in nc.m.functions:
        for bb in f.blocks:
            fix_block(bb)


def _get_nc():
    if "nc" not in _CACHE:
        nc = _build_nc()
        _split_multi_waits(nc)
        _CACHE["nc"] = nc
    return _CACHE["nc"]


def _get_runner():
    """Build (once) a cached PJRT executor for the SPMD bass program.

    Mirrors concourse.bass_utils.run_bass_kernel_spmd's axon path
    (bass2jax.run_bass_via_pjrt) but caches the jitted shard_map callable so
    repeated kernel() calls don't re-trace/re-compile (~10 s each)."""
    if "runner" in _CACHE:
        return _CACHE["runner"]

    import jax
    from jax.sharding import Mesh, PartitionSpec
    from jax.experimental.shard_map import shard_map
    from concourse import bass2jax

    nc = _get_nc()
    bass2jax.install_neuronx_cc_hook()

    partition_name = (nc.partition_id_tensor.name
                      if nc.partition_id_tensor else None)
    in_names, out_names, out_avals, zero_shapes = [], [], [], []
    for alloc in nc.m.functions[0].allocations:
        if not isinstance(alloc, mybir.MemoryLocationSet):
            continue
        name = alloc.memorylocations[0].name
        if alloc.kind == "ExternalInput":
            if name != partition_name:
                in_names.append(name)
        elif alloc.kind == "ExternalOutput":
            shape = tuple(alloc.tensor_shape)
            dtype = mybir.dt.np(alloc.dtype)
            out_names.append(name)
            out_avals.append(jax.core.ShapedArray(shape, dtype))
            zero_shapes.append((shape, dtype))
    n_params = len(in_names)
    all_in_names = list(in_names) + list(out_names)
    if partition_name is not None:
        all_in_names.append(partition_name)
    donate = tuple(range(n_params, n_params + len(out_names)))

    def _body(*args):
        operands = list(args)
        if partition_name is not None:
            operands.append(bass2jax.partition_id_tensor())
        outs = bass2jax._bass_exec_p.bind(
            *operands,
            out_avals=tuple(out_avals),
            in_names=tuple(all_in_names),
            out_names=tuple(out_names),
            lowering_input_output_aliases=(),
            sim_require_finite=True,
            sim_require_nnan=True,
            nc=nc,
        )
        return tuple(outs)

    devices = jax.devices()[:N_CORES]
    mesh = Mesh(np.asarray(devices), ("core",))
    n_io = n_params + len(out_names)
    sharded = jax.jit(
        shard_map(_body, mesh=mesh,
                  in_specs=(PartitionSpec("core"),) * n_io,
                  out_specs=(PartitionSpec("core"),) * len(out_names),
                  check_rep=False),
        donate_argnums=donate, keep_unused=True)

    runner = {
        "sharded": sharded,
        "in_names": in_names,
        "out_names": out_names,
        "zero_shapes": zero_shapes,
    }
    _CACHE["runner"] = runner
    return runner


def _shard_actions(actions):
    """actions [2048, 8, 256] int -> [16, 128, 256, 8] uint8, the concat of 8
    per-core shards (a-major transpose so each SBUF partition holds one `a`
    row; per-partition rows are 2 KiB contiguous for line-rate DMA loads).
    Values are 0..15 so uint8 is exact and shrinks the load 4x vs int32."""
    at = actions.reshape(N_CORES, F_PER_CORE, NUM_TYPES, NUM_ACTIONS)
    at = at.astype(np.uint8).transpose(0, 3, 1, 2)       # [8, 256a, 256f, 8t]
    return np.ascontiguousarray(at).reshape(
        N_CORES * 2, 128, F_PER_CORE, NUM_TYPES)


def _run_fallback(act_global):
    """Stock path via run_bass_kernel_spmd (works on native-NRT hosts too;
    re-jits per call, so only used if the cached PJRT runner path fails)."""
    from concourse.bass_utils import run_bass_kernel_spmd
    nc = _get_nc()
    in_maps = [{"actions_t": act_global[2 * c:2 * c + 2]}
               for c in range(N_CORES)]
    res = run_bass_kernel_spmd(nc, in_maps, core_ids=list(range(N_CORES)))
    return np.concatenate([r["out"] for r in res.results], axis=0)


def kernel(actions, base):
    actions = np.asarray(actions)
    base = np.asarray(base)
    assert actions.shape == (NUM_FRAMES, NUM_TYPES, NUM_ACTIONS), actions.shape
    act_global = _shard_actions(actions)
    try:
        r = _get_runner()
        assert r["in_names"] == ["actions_t"] and r["out_names"] == ["out"]
        (shape, dtype), = r["zero_shapes"]
        zeros = np.zeros((N_CORES * shape[0], *shape[1:]), dtype)
        out_global, = r["sharded"](act_global, zeros)
        out = np.asarray(out_global)
    except Exception:
        out = _run_fallback(act_global)
    out = out.reshape(NUM_FRAMES, NUM_ACTIONS, TOTAL)
    return out.astype(base.dtype, copy=False)
